# revision 1
# baseline (speedup 1.0000x reference)
"""Multi-head self-attention (B=8, N=1024, C=768, H=12) on 8 trn2 NeuronCores.

Sharding: data-parallel over batch — core b computes batch element b end to
end; weights are replicated. No collectives.

Per-core dataflow (all matmuls on TensorE, out = lhsT.T @ rhs, contraction on
the partition dim):

  1. qkv^T for Q,K in [c', n] layout:  lhsT = Wqkv^T k-tile, rhs = x^T k-tile.
     One [128,1024] PSUM tile per c'-tile (both 512-halves), bias fused into
     the PSUM->SBUF copy on DVE. DMA is prioritized so the pair-0 tiles
     (t=0 and t=6) land first and attention starts ~8us in.
  2. V in token-major per-head blocks [ones(64) | V_h] (128 cols per head):
     the 64 ones columns make the A@V matmul produce the softmax row-sums
     replicated across 64 partitions, so normalization needs no partition
     broadcast. V bias is skipped on-device: since softmax rows sum to 1, it
     folds into an adjusted proj bias bp' = b_proj + W_proj @ b_qkv[V]
     (host-computed).
  3. Per head h: S^T[m, n] = (K_h^T).T-stationary @ Q_h^T (K = d = 64).
     exp via ScalarE reading PSUM, writing SBUF (scale folded into the K
     projection host-side; max-subtraction skipped — scores are O(1) here and
     softmax is shift-invariant so the result is identical).
  4. AV runs one full head deferred, as two consecutive 8-matmul accumulation
     runs (g=0,1) per head — consecutive same-bank accumulation avoids the
     ~90ns/matmul weight-load handoff penalty that interleaved accumulation
     groups pay. Out rows 0:64 = row-sums (replicated), 64:128 = O_h^T.
  5. normalize: custom-DVE fast reciprocal on the replicated sums (PSUM base
     partition 0), then one tensor_mul into ouT[c, n] stacked across heads.
  6. proj is split: the j=0..4 k-tile contributions run as filler inside
     heads 10-11's slots (PE is otherwise ACT/exp-starved there), staged to
     SBUF with the bias added; only the j=5 contribution + final add + y DMA
     (bf16, spread over 3 DMA queues) remain after the last head.

Scheduling: attention slots are emitted per (head, m-tile): scores + exp,
with the previous head's AV runs, V production (head 0), remaining q/k tiles
(heads 1-9) and partial proj (heads 10-11) drained as PE filler inside the
ACT-bound slots. PSUM: 2x[128,1024] (scores + all filler units, queue-
rotated) + 4x[128,512] AV accumulators.

All matmul inputs are bf16 (fp32 accumulate); y is written bf16 and upcast
on host. Measured end-to-end error vs the fp32 reference ~2.4e-3
scale-relative.
"""

import numpy as np
import ml_dtypes

B, N, C = 8, 1024, 768
H, D = 12, 64
HB = 2 * D  # per-head V block width: [ones(64) | V_h(64)]
N_CORES = 8
P = 128
KT = C // P  # 6 contraction tiles
NT = N // P  # 8 token tiles
NQT = 2 * C // P  # 12 q/k c'-tiles; pair p uses tiles p and 6+p

_CACHE: dict = {}


def _build(cfg: dict):
    import concourse.bass as bass
    import concourse.bacc as bacc
    import concourse.mybir as mybir
    import concourse.tile as tile

    dt = mybir.dt
    f32 = dt.float32
    bf16 = dt.bfloat16

    nc = bacc.Bacc("TRN2", target_bir_lowering=False, debug=False,
                   num_devices=N_CORES)

    xT_d = nc.dram_tensor("xT", [C, N], bf16, kind="ExternalInput")
    wqkvT_d = nc.dram_tensor("wqkvT", [C, 3 * C], bf16, kind="ExternalInput")
    wprojT_d = nc.dram_tensor("wprojT", [C, C], bf16, kind="ExternalInput")
    bqk_d = nc.dram_tensor("bqk", [P, NQT], f32, kind="ExternalInput")
    bp_d = nc.dram_tensor("bp", [1, C], f32, kind="ExternalInput")
    ident_d = nc.dram_tensor("ident", [P, P], bf16, kind="ExternalInput")
    y_d = nc.dram_tensor("y", [N, C], bf16, kind="ExternalOutput")

    with tile.TileContext(nc, pool_alloc_mode="queue") as tc:
        with (
            tc.tile_pool(name="const", bufs=1) as cpool,
            tc.tile_pool(name="et", bufs=cfg["et_bufs"]) as etpool,
            tc.tile_pool(name="work", bufs=2) as workpool,
            tc.tile_pool(name="ps_s", bufs=2, space="PSUM") as ps_s,
            tc.tile_pool(name="ps_av", bufs=4, space="PSUM") as ps_av,
        ):
            # ---- resident loads: 8 batched transfers over 5 queues ----
            # Each dma_start costs ~600ns of trigger time serialized on the
            # issuing engine's queue, so inputs are batched into single
            # multi-dim-AP transfers: the pair-0 q/k weight slices (tensor
            # queue) and the two x halves (sync+vector) land first so the
            # PE can start ~5us in; everything else trickles underneath.
            # pair-0 weight slices: per k-tile, Q cols 0:128 and K cols
            # 768:896 — i.e. cols 0:128 of segments 0 and 1.
            # layout: [Q-slice k=0..5 | K-slice k=0..5], 128 cols each
            wqp0 = cpool.tile([P, KT * 4 * P], bf16, name="wqp0", tag="wqp0")
            src_w3 = wqkvT_d.ap().rearrange("(k p) n -> p k n", p=P)
            bqk = cpool.tile([P, NQT], f32, name="bqk", tag="bqk")
            bp = cpool.tile([1, C], f32, name="bp", tag="bp")
            # x^T as one tile, per-k transfers spread over three queues so
            # tiles land progressively (approx arrival order 2,4,3,0,5,1)
            xT1 = cpool.tile([P, KT * N], bf16, name="xT1", tag="xT1")
            def xdma(eng, k):
                eng.dma_start(xT1[:, k * N:(k + 1) * N],
                              xT_d.ap()[k * P:(k + 1) * P, :])
            wq1 = cpool.tile([P, KT * 3 * C], bf16, name="wq1", tag="wq1")
            wp1 = cpool.tile([P, KT * C], bf16, name="wp1", tag="wp1")
            dst_w = wq1[:].rearrange("p (k n) -> p k n", k=KT)
            def wqp0_dma(eng, blk, col0):
                eng.dma_start(
                    wqp0[:, blk * KT * P:(blk + 1) * KT * P]
                    .rearrange("p (k c) -> p k c", k=KT),
                    src_w3[:, :, col0:col0 + P])
            # sync queue: t0 slices, x0, x1, t1 slices
            wqp0_dma(nc.sync, 0, 0)
            xdma(nc.sync, 0)
            xdma(nc.sync, 1)
            wqp0_dma(nc.sync, 1, P)
            # scalar queue: x2, x3, t6+t7 slices, then bulk Q/K weights
            xdma(nc.scalar, 2)
            xdma(nc.scalar, 3)
            wqp0_dma(nc.scalar, 2, C)
            wqp0_dma(nc.scalar, 3, C + P)
            nc.scalar.dma_start(dst_w[:, :, 2 * C:3 * C],
                                src_w3[:, :, 2 * C:3 * C])
            nc.scalar.dma_start(dst_w[:, :, 0:2 * C], src_w3[:, :, 0:2 * C])
            # gpsimd queue: x4, biases, x5, proj weights
            xdma(nc.gpsimd, 4)
            nc.gpsimd.dma_start(bqk[:], bqk_d.ap())
            nc.gpsimd.dma_start(bp[:], bp_d.ap())
            xdma(nc.gpsimd, 5)
            nc.gpsimd.dma_start(
                wp1[:].rearrange("p (k n) -> p k n", k=KT),
                wprojT_d.ap().rearrange("(k p) n -> p k n", p=P))
            ident = cpool.tile([P, P], bf16, name="ident", tag="ident")
            nc.gpsimd.dma_start(ident[:], ident_d.ap())
            bp_b = cpool.tile([P, C], f32, name="bp_b", tag="bp_b")
            nc.gpsimd.partition_broadcast(bp_b[:], bp[:])
            # PE warm-up: junk matmuls on a zeroed tile during the DMA dead
            # zone, so the clock is ramped when real data lands (the PE runs
            # at ~1.2GHz until ~6us of continuous execution).
            zt = cpool.tile([P, 512], bf16, name="zt", tag="zt")
            nc.vector.memset(zt[:], 0.0)
            jp = ps_s.tile([P, N], f32, name="jp", tag="s")
            for _ in range(13):
                nc.tensor.matmul(jp[:, 0:512], zt[:, 0:P], zt[:],
                                 start=True, stop=True)

            def xT(k):
                return xT1[:, k * N:(k + 1) * N]

            def wq(k):
                return wq1[:, k * 3 * C:(k + 1) * 3 * C]

            def wp(k):
                return wp1[:, k * C:(k + 1) * C]

            # ---- tiles ----
            qkT = [cpool.tile([P, N], bf16, name=f"qkT{t}", tag=f"qkT{t}")
                   for t in range(NQT)]
            etbig = [cpool.tile([P, NT * 2 * N], bf16, name=f"etbig{i}",
                                tag=f"etbig{i}") for i in range(2)]
            vbig = cpool.tile([P, NT * H * HB], bf16, name="vbig",
                              tag="vbig")
            v = [vbig[:, nt * H * HB:(nt + 1) * H * HB] for nt in range(NT)]
            ouT = [cpool.tile([P, N], bf16, name=f"ouT{j}", tag=f"ouT{j}")
                   for j in range(KT)]
            ysb = [cpool.tile([P, C], bf16, name=f"ysb{nt}", tag=f"ysb{nt}")
                   for nt in range(NT)]

            # ---- filler units (each allocates one ps_s [128,1024] tile) ----
            def qk_unit(t):
                # Q^T or K^T tile t: [c'=128, n=1024], 2x6 accumulation runs.
                # Pair-0 tiles (t=0,6) read the early wqp0 slices.
                def w(k):
                    blk = {0: 0, 1: 1, KT: 2, KT + 1: 3}.get(t)
                    if blk is not None:
                        return wqp0[:, (blk * KT + k) * P:(blk * KT + k + 1) * P]
                    s, c0 = (0, t * P) if t < KT else (1, (t - KT) * P)
                    return wq(k)[:, s * C + c0:s * C + c0 + P]
                # k in DMA arrival order for the very first unit
                ks = ([2, 4, 3, 0, 5, 1] if t in (0, 1, KT, KT + 1)
                      else list(range(KT)))
                pm = ps_s.tile([P, N], f32, name="mm", tag="s")
                for g in range(2):
                    for i, k in enumerate(ks):
                        nc.tensor.matmul(
                            pm[:, g * 512:(g + 1) * 512],
                            w(k),
                            xT(k)[:, g * 512:(g + 1) * 512],
                            start=(i == 0), stop=(i == KT - 1),
                        )
                if t in (0, KT):  # halve so first scores unblock earlier
                    for g in range(2):
                        sl = slice(g * 512, (g + 1) * 512)
                        nc.vector.tensor_scalar_add(
                            qkT[t][:, sl], pm[:, sl], bqk[:, t:t + 1])
                else:
                    nc.vector.tensor_scalar_add(qkT[t][:], pm[:],
                                                bqk[:, t:t + 1])

            def v_half(nt, half):
                # V for token tile nt, heads 0-7 (half 0) or 8-11 (half 1);
                # per-head blocks [ones(64) | V_h(64)]. Split keeps head-0's
                # slots small enough that the PE clock stays ramped.
                dst = v[nt].rearrange("p (h c) -> p h c", c=HB)
                if half == 0:
                    nc.vector.memset(dst[:, :, 0:D], 1.0)
                off, width, h0, h1 = ((0, 512, 0, 8) if half == 0
                                      else (512, 256, 8, 12))
                pm = ps_s.tile([P, N], f32, name="mm", tag="s")
                for k in range(KT):
                    nc.tensor.matmul(
                        pm[:, 0:width],
                        xT(k)[:, nt * P:(nt + 1) * P],
                        wq(k)[:, 2 * C + off:2 * C + off + width],
                        start=(k == 0), stop=(k == KT - 1),
                    )
                srcv = pm[:, 0:width].rearrange("p (h d) -> p h d", d=D)
                nc.vector.tensor_copy(dst[:, h0:h1, D:HB], srcv[:])

            def proj_partial(nt):
                # y[nt] partial: k-tiles j=0..4, staged to SBUF with bias
                pm = ps_s.tile([P, N], f32, name="mm", tag="s")
                for j in range(KT - 1):
                    for off, width in ((0, 512), (512, 256)):
                        nc.tensor.matmul(
                            pm[:, off:off + width],
                            ouT[j][:, nt * P:(nt + 1) * P],
                            wp(j)[:, off:off + width],
                            start=(j == 0), stop=(j == KT - 2),
                        )
                nc.vector.tensor_add(ysb[nt][:], pm[:, 0:C], bp_b[:])

            # ---- attention pieces ----
            class PairState:
                def __init__(self, p):
                    self.p = p
                    self.o_ps = {}

            def score_exp(st, mt, g):
                # both heads of the pair, back-to-back at tile rows 0/64 —
                # the PE runs the two K=64 quadrant matmuls concurrently
                qt = qkT[st.p]
                kt = qkT[NQT // 2 + st.p]
                sp = ps_s.tile([P, N], f32, name="sp", tag="s")
                for par in range(2):
                    o = par * D
                    nc.tensor.matmul(
                        sp[:, par * 512:(par + 1) * 512],
                        kt[o:o + D, mt * P:(mt + 1) * P],
                        qt[o:o + D, g * 512:(g + 1) * 512],
                        start=True, stop=True,
                    )
                et = etbig[st.p % 2][:, (mt * 2 + g) * N:(mt * 2 + g + 1) * N]
                nc.scalar.activation(
                    et, sp[:], bass.mybir.ActivationFunctionType.Exp)

            def av_run(st, par, g):
                # one consecutive 8-matmul accumulation run into one bank
                if (par, g) not in st.o_ps:
                    st.o_ps[(par, g)] = ps_av.tile([P, 512], f32,
                                                   name="o_ps", tag="av")
                h = 2 * st.p + par
                eb = etbig[st.p % 2]
                for mt in range(NT):
                    nc.tensor.matmul(
                        st.o_ps[(par, g)][:],
                        v[mt][:, h * HB:(h + 1) * HB],
                        eb[:, (mt * 2 + g) * N + par * 512:
                           (mt * 2 + g) * N + (par + 1) * 512],
                        start=(mt == 0), stop=(mt == NT - 1),
                    )

            def normalize(st, par):
                # sums are replicated on partitions 0:64 of o_ps; O^T on
                # 64:128. recip reads PSUM at base partition 0 (the custom
                # DVE op mis-reads PSUM only at base partition 64).
                for g in range(2):
                    sl = slice(g * 512, (g + 1) * 512)
                    rb = workpool.tile([D, 512], f32, name="rb", tag="rb")
                    ops = st.o_ps[(par, g)]
                    nc.vector.reciprocal_approx_fast(rb[:], ops[0:D, :])
                    nc.vector.tensor_mul(
                        ouT[st.p][par * D:(par + 1) * D, sl],
                        ops[D:P, :], rb[:])

            # ---- filler schedule ----
            # pair 0: the 8 V-first-halves (g0 slots) + q/k tiles 1,7 (g1
            # slots). pairs 1-4: one q/k tile at slots (2,g0)/(6,g0) and V
            # second-halves at (1,g0)/(5,g0). pair 5: proj partials after
            # normalize(pair 4) lands at slot (4,g1).
            slot_fill: dict = {}
            for mt in range(NT):
                slot_fill[(0, mt, 0)] = (lambda mt=mt: v_half(mt, 0))
            slot_fill[(0, 6, 1)] = (lambda: qk_unit(1))
            slot_fill[(0, 7, 1)] = (lambda: qk_unit(7))
            for pq, (ta, tb) in zip(range(1, 5),
                                    [(2, 8), (3, 9), (4, 10), (5, 11)]):
                slot_fill[(pq, 2, 0)] = (lambda t=ta: qk_unit(t))
                slot_fill[(pq, 6, 0)] = (lambda t=tb: qk_unit(t))
            for i in range(8):
                p_, s_ = 1 + i // 2, (1, 0) if i % 2 == 0 else (5, 0)
                slot_fill[(p_,) + s_] = (lambda nt=i: v_half(nt, 1))
            pp_slots = [(5, 4, 1), (5, 5, 0), (5, 5, 1), (5, 6, 0),
                        (5, 6, 1), (5, 7, 0), (5, 7, 1)]
            for sl_, nt in zip(pp_slots, range(7)):
                slot_fill[sl_] = (lambda nt=nt: proj_partial(nt))

            # pair-0 q/k tiles up front — unblocks attention immediately
            qk_unit(0)
            qk_unit(6)

            # ---- main loop over head pairs: each (mt, g) slot runs the
            # pair's two quadrant-concurrent score matmuls + exp, with the
            # previous pair's four AV runs (g1 slots of mt 0-3), its
            # normalizes (mt 4 g1) and filler drained inside the slots.
            prev = None
            for p in range(H // 2):
                st = PairState(p)
                for mt in range(NT):
                    for g in range(2):
                        score_exp(st, mt, g)
                        if prev is not None and g == 1:
                            if mt < 4:
                                av_run(prev, mt // 2, mt % 2)
                            elif mt == 4:
                                normalize(prev, 0)
                                normalize(prev, 1)
                        u = slot_fill.pop((p, mt, g), None)
                        if u is not None:
                            u()
                prev = st
            for par in range(2):
                for g in range(2):
                    av_run(prev, par, g)
            normalize(prev, 0)
            normalize(prev, 1)
            proj_partial(NT - 1)  # covers the normalize DVE latency

            # ---- tail: j=5 proj contribution + final add + y DMA.
            # Alternate two drain paths so the post-normalize serial chain
            # splits across DVE (tensor_add) and PE+ACT (identity-matmul
            # accumulate + Copy) instead of one 8-deep DVE chain.
            dma_engines = [nc.sync, nc.scalar, nc.gpsimd]
            for nt in range(NT):
                ev = nt % 2 == 0
                pm = ps_s.tile([P, N], f32, name="mm", tag="s")
                for off, width in ((0, 512), (512, 256)):
                    nc.tensor.matmul(
                        pm[:, off:off + width],
                        ouT[KT - 1][:, nt * P:(nt + 1) * P],
                        wp(KT - 1)[:, off:off + width],
                        start=True, stop=not ev,
                    )
                    if ev:
                        nc.tensor.matmul(
                            pm[:, off:off + width], ident[:],
                            ysb[nt][:, off:off + width],
                            start=False, stop=True)
                yb = workpool.tile([P, C], bf16, name="yb", tag="yb",
                                   bufs=4)
                if ev:
                    nc.scalar.copy(yb[:], pm[:, 0:C])
                else:
                    nc.vector.tensor_add(yb[:], pm[:, 0:C], ysb[nt][:])
                dma_engines[nt % 3].dma_start(
                    y_d.ap()[nt * P:(nt + 1) * P, :], yb[:])

    nc.compile()
    return nc


DEFAULT_CFG = dict(et_bufs=16)


def _host_prep(x, W_qkv, b_qkv, W_proj, b_proj, cfg):
    """Shard + lay out host-side numpy inputs per core."""
    scale = 1.0 / np.sqrt(D)
    wqkvT = np.ascontiguousarray(W_qkv.T).astype(np.float32)
    # fold the 1/sqrt(D) score scale into the K projection (cols C:2C)
    wqkvT[:, C:2 * C] *= scale
    wqkvT = wqkvT.astype(ml_dtypes.bfloat16)
    wprojT = np.ascontiguousarray(W_proj.T).astype(ml_dtypes.bfloat16)
    bqk_f = b_qkv[:2 * C].astype(np.float32).copy()
    bqk_f[C:2 * C] *= scale
    bqk = np.ascontiguousarray(bqk_f.reshape(NQT, P).T).astype(np.float32)
    bp_eff = (b_proj.astype(np.float64)
              + W_proj.astype(np.float64) @ b_qkv[2 * C:].astype(np.float64))
    bp = bp_eff.astype(np.float32).reshape(1, C)
    ident = np.eye(P, dtype=ml_dtypes.bfloat16)
    in_maps = []
    for b in range(N_CORES):
        xT = np.ascontiguousarray(x[b].T).astype(ml_dtypes.bfloat16)
        in_maps.append({"xT": xT, "wqkvT": wqkvT, "wprojT": wprojT,
                        "bqk": bqk, "bp": bp, "ident": ident})
    return in_maps


def get_nc(cfg=None):
    cfg = dict(DEFAULT_CFG, **(cfg or {}))
    key = tuple(sorted(cfg.items()))
    if key not in _CACHE:
        _CACHE[key] = _build(cfg)
    return _CACHE[key]


def run(inputs, cfg=None, **run_kwargs):
    from concourse import bass_utils

    cfg = dict(DEFAULT_CFG, **(cfg or {}))
    nc = get_nc(cfg)
    in_maps = _host_prep(inputs["x"], inputs["W_qkv"], inputs["b_qkv"],
                         inputs["W_proj"], inputs["b_proj"], cfg)
    res = bass_utils.run_bass_kernel_spmd(
        nc, in_maps, core_ids=list(range(N_CORES)), **run_kwargs)
    out = np.stack([res.results[b]["y"].astype(np.float32)
                    for b in range(N_CORES)], axis=0)
    return out, res


def kernel(**inputs) -> np.ndarray:
    inputs = {k: np.asarray(v) for k, v in inputs.items()}
    out, _ = run(inputs)
    return out



# revision 5
# speedup vs baseline: 1.0399x; 1.0399x over previous
"""Multi-head self-attention (B=8, N=1024, C=768, H=12) on 8 trn2 NeuronCores.

Sharding: data-parallel over batch — core b computes batch element b end to
end; weights are replicated. No collectives.

Per-core dataflow (all matmuls on TensorE, out = lhsT.T @ rhs, contraction on
the partition dim):

  1. qkv^T for Q,K in [c', n] layout:  lhsT = Wqkv^T k-tile, rhs = x^T k-tile.
     Emitted as fine-grained units: one 6-matmul accumulation run per
     (c'-tile, 512-col half) into a [128,512] PSUM bank, bias fused into the
     PSUM->SBUF copy on DVE. Pair-0 tiles (t=0,6) read early wqp0 slices.
  2. V in token-major per-head blocks [ones(64) | V_h] (128 cols per head):
     the 64 ones columns make the A@V matmul produce the softmax row-sums
     replicated across 64 partitions, so normalization needs no partition
     broadcast. V bias is skipped on-device: since softmax rows sum to 1, it
     folds into an adjusted proj bias bp' = b_proj + W_proj @ b_qkv[V]
     (host-computed).
  3. Per head pair p, slot s -> (mt, g): S^T[m, n] = (K_h^T) @ Q_h^T for both
     heads concurrently in the two PE row-quadrants (K = d = 64). exp via
     ScalarE reading PSUM (3-deep [128,1024] rotation so the PE can run ~2
     slots ahead of ACT), writing SBUF bf16 (scale folded into the K
     projection host-side; max-subtraction skipped — scores are O(1) and
     softmax is shift-invariant).
  4. AV: one 8-matmul consecutive same-bank accumulation run per (head, g)
     into a [128,512] bank from a small 2-deep PSUM pool, normalized eagerly
     (custom-DVE fast reciprocal on the replicated sums at PSUM base
     partition 0, then tensor_mul into ouT[c, n]) so the bank frees in ~2
     slots.  Out rows 0:64 = row-sums, 64:128 = O_h^T.
  5. proj in three stages so the last head's serial tail is short:
     projA = j=0..2 k-tiles -> ysb[nt] (+bias), run inside pairs 3-4;
     projB = j=3..4 + identity-matmul merge of ysb, run inside pair 5;
     tail  = j=5 + merge + y DMA (bf16, spread over 3 DMA queues).
  6. Pair 5 runs its slots g-major (all g=0 then all g=1) so its AV(g=0),
     normalize, and the nt=0..3 tail units overlap the g=1 exps; only the
     g=1 AV/normalize/tail remains after the last exp.

Scheduling: after each score+exp emission, filler units (each one PSUM-bank
accumulation run + one DVE drain) are drained from a per-pair list by
cumulative time-budget pacing with per-unit earliest-slot constraints
matching DMA arrival and dependency readiness. The ACT exp table is
preloaded via a dummy 1-col exp during the DMA dead zone.

All matmul inputs are bf16 (fp32 accumulate); y is written bf16 and upcast
on host.
"""

import numpy as np
import ml_dtypes

B, N, C = 8, 1024, 768
H, D = 12, 64
HB = 2 * D  # per-head V block width: [ones(64) | V_h(64)]
N_CORES = 8
P = 128
KT = C // P  # 6 contraction tiles
NT = N // P  # 8 token tiles
NQT = 2 * C // P  # 12 q/k c'-tiles; pair p uses tiles p and 6+p

_CACHE: dict = {}

MM512 = 215  # ns, warm 512-col bf16 matmul issue-to-issue
MM256 = 110


def _build(cfg: dict):
    import concourse.bass as bass
    import concourse.bacc as bacc
    import concourse.mybir as mybir
    import concourse.tile as tile

    dt = mybir.dt
    f32 = dt.float32
    bf16 = dt.bfloat16

    nc = bacc.Bacc("TRN2", target_bir_lowering=False, debug=False,
                   num_devices=N_CORES)

    xT_d = nc.dram_tensor("xT", [C, N], bf16, kind="ExternalInput")
    wqkvT_d = nc.dram_tensor("wqkvT", [C, 3 * C], bf16, kind="ExternalInput")
    wprojT_d = nc.dram_tensor("wprojT", [C, C], bf16, kind="ExternalInput")
    bqk_d = nc.dram_tensor("bqk", [P, NQT], f32, kind="ExternalInput")
    bp_d = nc.dram_tensor("bp", [1, C], f32, kind="ExternalInput")
    ident_d = nc.dram_tensor("ident", [P, P], bf16, kind="ExternalInput")
    y_d = nc.dram_tensor("y", [N, C], bf16, kind="ExternalOutput")

    with tile.TileContext(nc, pool_alloc_mode="queue") as tc:
        with (
            tc.tile_pool(name="const", bufs=1) as cpool,
            tc.tile_pool(name="work", bufs=2) as workpool,
            tc.tile_pool(name="ps_s", bufs=3, space="PSUM") as ps_s,
            tc.tile_pool(name="ps_f", bufs=2, space="PSUM") as ps_f,
        ):
            # ---- resident loads, batched multi-dim-AP transfers over the
            # engine DMA queues; pair-0 q/k weight slices + x tiles first so
            # attention can start ~6us in.
            wqp0 = cpool.tile([P, KT * 4 * P], bf16, name="wqp0", tag="wqp0")
            src_w3 = wqkvT_d.ap().rearrange("(k p) n -> p k n", p=P)
            bqk = cpool.tile([P, NQT], f32, name="bqk", tag="bqk")
            bp = cpool.tile([1, C], f32, name="bp", tag="bp")
            xT1 = cpool.tile([P, KT * N], bf16, name="xT1", tag="xT1")

            def xdma(eng, k):
                eng.dma_start(xT1[:, k * N:(k + 1) * N],
                              xT_d.ap()[k * P:(k + 1) * P, :])
            wq1 = cpool.tile([P, KT * 3 * C], bf16, name="wq1", tag="wq1")
            wp1 = cpool.tile([P, KT * C], bf16, name="wp1", tag="wp1")
            dst_w = wq1[:].rearrange("p (k n) -> p k n", k=KT)

            def wqp0_dma(eng, blk, col0):
                eng.dma_start(
                    wqp0[:, blk * KT * P:(blk + 1) * KT * P]
                    .rearrange("p (k c) -> p k c", k=KT),
                    src_w3[:, :, col0:col0 + P])
            zt = cpool.tile([P, 512], bf16, name="zt", tag="zt")
            nc.vector.memset(zt[:], 0.0)
            # sync queue: Q0/K0 slices, x0, x1, Q1/K1 slices
            wqp0_dma(nc.sync, 0, 0)
            wqp0_dma(nc.sync, 2, C)
            xdma(nc.sync, 0)
            xdma(nc.sync, 1)
            wqp0_dma(nc.sync, 1, P)
            wqp0_dma(nc.sync, 3, C + P)
            # scalar queue: x2, x3, bulk V wts, exp-table preload (dummy
            # 1-col exp during the transfer dead zone), bulk Q/K wts
            xdma(nc.scalar, 2)
            xdma(nc.scalar, 3)
            nc.scalar.dma_start(dst_w[:, :, 2 * C:3 * C],
                                src_w3[:, :, 2 * C:3 * C])
            dume = cpool.tile([P, 1], bf16, name="dume", tag="dume")
            nc.scalar.activation(dume[:], zt[:, 0:1],
                                 bass.mybir.ActivationFunctionType.Exp)
            nc.scalar.dma_start(dst_w[:, :, 0:2 * C], src_w3[:, :, 0:2 * C])
            # gpsimd queue: x4, biases, x5, proj weights
            xdma(nc.gpsimd, 4)
            nc.gpsimd.dma_start(bqk[:], bqk_d.ap())
            nc.gpsimd.dma_start(bp[:], bp_d.ap())
            xdma(nc.gpsimd, 5)
            nc.gpsimd.dma_start(
                wp1[:].rearrange("p (k n) -> p k n", k=KT),
                wprojT_d.ap().rearrange("(k p) n -> p k n", p=P))
            ident = cpool.tile([P, P], bf16, name="ident", tag="ident")
            nc.gpsimd.dma_start(ident[:], ident_d.ap())
            bp_b = cpool.tile([P, C], f32, name="bp_b", tag="bp_b")
            nc.gpsimd.partition_broadcast(bp_b[:], bp[:])
            # PE warm-up: junk matmuls on the zeroed tile during the DMA dead
            # zone so the HAM clock is ramped when real data lands.
            jp = ps_s.tile([P, N], f32, name="jp", tag="s")
            for _ in range(13):
                nc.tensor.matmul(jp[:, 0:512], zt[:, 0:P], zt[:],
                                 start=True, stop=True)

            def xT(k):
                return xT1[:, k * N:(k + 1) * N]

            def wq(k):
                return wq1[:, k * 3 * C:(k + 1) * 3 * C]

            def wp(k):
                return wp1[:, k * C:(k + 1) * C]

            # ---- persistent SBUF tiles ----
            qkT = [cpool.tile([P, N], bf16, name=f"qkT{t}", tag=f"qkT{t}")
                   for t in range(NQT)]
            etbig = [cpool.tile([P, NT * 2 * N], bf16, name=f"etbig{i}",
                                tag=f"etbig{i}") for i in range(2)]
            vbig = cpool.tile([P, NT * H * HB], bf16, name="vbig",
                              tag="vbig")
            v = [vbig[:, nt * H * HB:(nt + 1) * H * HB] for nt in range(NT)]
            ouT = [cpool.tile([P, N], bf16, name=f"ouT{j}", tag=f"ouT{j}")
                   for j in range(KT)]
            ysb = [cpool.tile([P, C], bf16, name=f"ysb{nt}", tag=f"ysb{nt}")
                   for nt in range(NT)]

            # ---- fine-grained filler units ----
            def qk_run(t, g):
                # Q^T or K^T tile t, 512-col half g: 6-matmul accumulation
                # run; pair-0 tiles (0,1,6,7) read the early wqp0 slices.
                def w(k):
                    blk = {0: 0, 1: 1, KT: 2, KT + 1: 3}.get(t)
                    if blk is not None:
                        return wqp0[:, (blk * KT + k) * P:
                                    (blk * KT + k + 1) * P]
                    s_, c0 = (0, t * P) if t < KT else (1, (t - KT) * P)
                    return wq(k)[:, s_ * C + c0:s_ * C + c0 + P]
                ks = ([2, 4, 3, 0, 5, 1] if t in (0, 1, KT, KT + 1)
                      else list(range(KT)))
                pm = ps_f.tile([P, 512], f32, name="fm", tag="f")
                sl = slice(g * 512, (g + 1) * 512)
                for i, k in enumerate(ks):
                    nc.tensor.matmul(pm[:], w(k), xT(k)[:, sl],
                                     start=(i == 0), stop=(i == KT - 1))
                nc.vector.tensor_scalar_add(qkT[t][:, sl], pm[:],
                                            bqk[:, t:t + 1])

            def v_half(nt, half):
                # V for token tile nt, heads 0-7 (half 0) or 8-11 (half 1);
                # per-head blocks [ones(64) | V_h(64)].
                dst = v[nt].rearrange("p (h c) -> p h c", c=HB)
                if half == 0:
                    nc.vector.memset(dst[:, :, 0:D], 1.0)
                off, width, h0, h1 = ((0, 512, 0, 8) if half == 0
                                      else (512, 256, 8, 12))
                pm = ps_f.tile([P, 512], f32, name="fm", tag="f")
                for k in range(KT):
                    nc.tensor.matmul(
                        pm[:, 0:width],
                        xT(k)[:, nt * P:(nt + 1) * P],
                        wq(k)[:, 2 * C + off:2 * C + off + width],
                        start=(k == 0), stop=(k == KT - 1),
                    )
                srcv = pm[:, 0:width].rearrange("p (h d) -> p h d", d=D)
                nc.vector.tensor_copy(dst[:, h0:h1, D:HB], srcv[:])

            def av_unit(prev, par, g):
                # one consecutive 8-matmul accumulation run + eager
                # normalize: sums replicated on partitions 0:64, O^T on
                # 64:128; recip reads PSUM at base partition 0.
                h = 2 * prev.p + par
                eb = etbig[prev.p % 2]
                pm = ps_f.tile([P, 512], f32, name="fm", tag="f")
                for i in range(NT):
                    s_idx = prev.slot_of(i, g)
                    nc.tensor.matmul(
                        pm[:],
                        v[i][:, h * HB:(h + 1) * HB],
                        eb[:, s_idx * N + par * 512:
                           s_idx * N + (par + 1) * 512],
                        start=(i == 0), stop=(i == NT - 1),
                    )
                rb = workpool.tile([D, 512], f32, name="rb", tag="rb")
                nc.vector.reciprocal_approx_fast(rb[:], pm[0:D, :])
                nc.vector.tensor_mul(
                    ouT[prev.p][par * D:(par + 1) * D,
                                g * 512:(g + 1) * 512],
                    pm[D:P, :], rb[:])

            def projA(nt, off, width):
                # ysb[nt] <- sum_{j=0..2} ouT[j]^T @ wp[j] + bias
                pm = ps_f.tile([P, 512], f32, name="fm", tag="f")
                for j in range(3):
                    nc.tensor.matmul(
                        pm[:, 0:width],
                        ouT[j][:, nt * P:(nt + 1) * P],
                        wp(j)[:, off:off + width],
                        start=(j == 0), stop=(j == 2),
                    )
                nc.vector.tensor_add(ysb[nt][:, off:off + width],
                                     pm[:, 0:width],
                                     bp_b[:, off:off + width])

            def projB(nt, off, width):
                # ysb[nt] <- ysb[nt] + sum_{j=3..4}, merged via identity mm
                pm = ps_f.tile([P, 512], f32, name="fm", tag="f")
                for j in (3, 4):
                    nc.tensor.matmul(
                        pm[:, 0:width],
                        ouT[j][:, nt * P:(nt + 1) * P],
                        wp(j)[:, off:off + width],
                        start=(j == 3), stop=False,
                    )
                nc.tensor.matmul(pm[:, 0:width], ident[:],
                                 ysb[nt][:, off:off + width],
                                 start=False, stop=True)
                nc.vector.tensor_copy(ysb[nt][:, off:off + width],
                                      pm[:, 0:width])

            dma_engines = [nc.sync, nc.scalar, nc.gpsimd]

            def tail_unit(nt):
                # j=5 contribution + merge with ysb + y DMA. Alternate the
                # merge between PE+ACT (identity mm + copy) and DVE (add).
                ev = nt % 2 == 0
                pm = ps_f.tile([P, 512], f32, name="fm", tag="f")
                pm2 = ps_f.tile([P, 512], f32, name="fm", tag="f")
                for ps, off, width in ((pm, 0, 512), (pm2, 512, 256)):
                    nc.tensor.matmul(
                        ps[:, 0:width],
                        ouT[KT - 1][:, nt * P:(nt + 1) * P],
                        wp(KT - 1)[:, off:off + width],
                        start=True, stop=not ev,
                    )
                    if ev:
                        nc.tensor.matmul(
                            ps[:, 0:width], ident[:],
                            ysb[nt][:, off:off + width],
                            start=False, stop=True)
                yb = workpool.tile([P, C], bf16, name="yb", tag="yb",
                                   bufs=4)
                if ev:
                    nc.scalar.copy(yb[:, 0:512], pm[:])
                    nc.scalar.copy(yb[:, 512:C], pm2[:, 0:256])
                else:
                    nc.vector.tensor_add(yb[:, 0:512], pm[:],
                                         ysb[nt][:, 0:512])
                    nc.vector.tensor_add(yb[:, 512:C], pm2[:, 0:256],
                                         ysb[nt][:, 512:C])
                dma_engines[nt % 3].dma_start(
                    y_d.ap()[nt * P:(nt + 1) * P, :], yb[:])

            # ---- attention pieces ----
            class PairState:
                def __init__(self, p):
                    self.p = p
                    self.gmajor = (p == H // 2 - 1)

                def slot_of(self, mt, g):
                    return g * NT + mt if self.gmajor else 2 * mt + g

            def score_exp(st, mt, g):
                qt = qkT[st.p]
                kt = qkT[NQT // 2 + st.p]
                sp = ps_s.tile([P, N], f32, name="sp", tag="s")
                for par in range(2):
                    o = par * D
                    nc.tensor.matmul(
                        sp[:, par * 512:(par + 1) * 512],
                        kt[o:o + D, mt * P:(mt + 1) * P],
                        qt[o:o + D, g * 512:(g + 1) * 512],
                        start=True, stop=True,
                    )
                s_idx = st.slot_of(mt, g)
                et = etbig[st.p % 2][:, s_idx * N:(s_idx + 1) * N]
                nc.scalar.activation(
                    et, sp[:], bass.mybir.ActivationFunctionType.Exp)

            # ---- per-pair filler unit lists: (min_slot, cost_ns, fn) ----
            NPAIR = H // 2
            units: list = [[] for _ in range(NPAIR)]
            UQK = 6 * MM512 + 80
            UV0 = 6 * MM512 + 80
            UV1 = 6 * MM256 + 80
            UAV = 8 * MM512 + 80
            UPA = {512: 3 * MM512 + 80, 256: 3 * MM256 + 80}
            UPB = {512: 3 * MM512 + 80, 256: 3 * MM256 + 80}
            UTL = 2 * MM512 + 2 * MM256 + 80

            # pair 0: the pair-1 qk tiles first (they read early wqp0
            # slices, x-gated only — they bridge the V-weight arrival gap),
            # then V half0 (V weights land ~6us), then vh1 x2.
            units[0].append((0, UQK, lambda: qk_run(1, 0)))
            units[0].append((0, UQK, lambda: qk_run(7, 0)))
            units[0].append((1, UQK, lambda: qk_run(1, 1)))
            units[0].append((2, UQK, lambda: qk_run(7, 1)))
            for nt in range(NT):
                units[0].append((nt + 3, UV0, lambda nt=nt: v_half(nt, 0)))
            units[0].append((11, UV1, lambda: v_half(0, 1)))
            units[0].append((12, UV1, lambda: v_half(1, 1)))
            # pairs 1-4: AV(prev) x4 from slot 2, vh1, next-pair qk tiles,
            # projA in pairs 3-4.
            vh1_sched = {1: [2, 3, 4], 2: [5, 6, 7], 3: [], 4: []}
            for p in range(1, 5):
                for i, (par, g) in enumerate(
                        ((0, 0), (1, 0), (0, 1), (1, 1))):
                    units[p].append((2 + i, UAV,
                                     lambda par=par, g=g: ("av", par, g)))
                for nt in vh1_sched[p]:
                    units[p].append((0, UV1, lambda nt=nt: v_half(nt, 1)))
                ta, tb = p + 1, KT + p + 1
                for g in range(2):
                    units[p].append((7, UQK, lambda t=ta, g=g: qk_run(t, g)))
                    units[p].append((9, UQK, lambda t=tb, g=g: qk_run(t, g)))
            for nt in range(4):
                units[3].append((8, UPA[512],
                                 lambda nt=nt: projA(nt, 0, 512)))
                units[3].append((8, UPA[256],
                                 lambda nt=nt: projA(nt, 512, 256)))
            for nt in range(4, NT):
                units[4].append((7, UPA[512],
                                 lambda nt=nt: projA(nt, 0, 512)))
                units[4].append((7, UPA[256],
                                 lambda nt=nt: projA(nt, 512, 256)))
            # pair 5 (g-major slots): AV(p4) x4, projB x8, AV(p5,g0) after
            # slot 8, then the nt 0-3 tails.
            for i, (par, g) in enumerate(((0, 0), (1, 0), (0, 1), (1, 1))):
                units[5].append((2 + i, UAV,
                                 lambda par=par, g=g: ("av", par, g)))
            for nt in range(NT):
                units[5].append((6 if nt < 4 else 7, UPB[512],
                                 lambda nt=nt: projB(nt, 0, 512)))
                units[5].append((6 if nt < 4 else 7, UPB[256],
                                 lambda nt=nt: projB(nt, 512, 256)))
            units[5].append((10, UAV, lambda: ("av5", 0, 0)))
            units[5].append((11, UAV, lambda: ("av5", 1, 0)))
            for nt in range(4):
                units[5].append((13, UTL, lambda nt=nt: tail_unit(nt)))

            # pair-0 q/k tiles up front — g0 halves first so the first
            # score slot unblocks as early as possible
            qk_run(0, 0)
            qk_run(6, 0)
            qk_run(0, 1)
            qk_run(6, 1)

            # ---- main loop ----
            prev = None
            cur = None
            for p in range(NPAIR):
                cur = PairState(p)
                ulist = units[p]
                total = sum(c for _, c, _ in ulist) + 16 * MM512
                spent = 0
                for s in range(16):
                    if cur.gmajor:
                        g, mt = divmod(s, NT)
                    else:
                        mt, g = divmod(s, 2)
                    score_exp(cur, mt, g)
                    spent += MM512
                    budget = total * (s + 1) // 16
                    while ulist:
                        idx = next((i for i, u in enumerate(ulist)
                                    if u[0] <= s), None)
                        if idx is None or spent > budget:
                            break
                        _, c, fn = ulist.pop(idx)
                        r = fn()
                        if isinstance(r, tuple):
                            if r[0] == "av":
                                av_unit(prev, r[1], r[2])
                            else:
                                av_unit(cur, r[1], r[2])
                        spent += c
                for _, c, fn in ulist:
                    r = fn()
                    if isinstance(r, tuple):
                        if r[0] == "av":
                            av_unit(prev, r[1], r[2])
                        else:
                            av_unit(cur, r[1], r[2])
                prev = cur

            # ---- tail: pair-5 g=1 AV + normalize + nt 4-7 tails ----
            for par in range(2):
                av_unit(prev, par, 1)
            for nt in range(4, NT):
                tail_unit(nt)

    nc.compile()
    return nc


DEFAULT_CFG = dict()


def _host_prep(x, W_qkv, b_qkv, W_proj, b_proj, cfg):
    """Shard + lay out host-side numpy inputs per core."""
    scale = 1.0 / np.sqrt(D)
    wqkvT = np.ascontiguousarray(W_qkv.T).astype(np.float32)
    # fold the 1/sqrt(D) score scale into the K projection (cols C:2C)
    wqkvT[:, C:2 * C] *= scale
    wqkvT = wqkvT.astype(ml_dtypes.bfloat16)
    wprojT = np.ascontiguousarray(W_proj.T).astype(ml_dtypes.bfloat16)
    bqk_f = b_qkv[:2 * C].astype(np.float32).copy()
    bqk_f[C:2 * C] *= scale
    bqk = np.ascontiguousarray(bqk_f.reshape(NQT, P).T).astype(np.float32)
    bp_eff = (b_proj.astype(np.float64)
              + W_proj.astype(np.float64) @ b_qkv[2 * C:].astype(np.float64))
    bp = bp_eff.astype(np.float32).reshape(1, C)
    ident = np.eye(P, dtype=ml_dtypes.bfloat16)
    in_maps = []
    for b in range(N_CORES):
        xT = np.ascontiguousarray(x[b].T).astype(ml_dtypes.bfloat16)
        in_maps.append({"xT": xT, "wqkvT": wqkvT, "wprojT": wprojT,
                        "bqk": bqk, "bp": bp, "ident": ident})
    return in_maps


def get_nc(cfg=None):
    cfg = dict(DEFAULT_CFG, **(cfg or {}))
    key = tuple(sorted(cfg.items()))
    if key not in _CACHE:
        _CACHE[key] = _build(cfg)
    return _CACHE[key]


def run(inputs, cfg=None, **run_kwargs):
    from concourse import bass_utils

    cfg = dict(DEFAULT_CFG, **(cfg or {}))
    nc = get_nc(cfg)
    in_maps = _host_prep(inputs["x"], inputs["W_qkv"], inputs["b_qkv"],
                         inputs["W_proj"], inputs["b_proj"], cfg)
    res = bass_utils.run_bass_kernel_spmd(
        nc, in_maps, core_ids=list(range(N_CORES)), **run_kwargs)
    out = np.stack([res.results[b]["y"].astype(np.float32)
                    for b in range(N_CORES)], axis=0)
    return out, res


def kernel(**inputs) -> np.ndarray:
    inputs = {k: np.asarray(v) for k, v in inputs.items()}
    out, _ = run(inputs)
    return out


# revision 6
# speedup vs baseline: 1.1038x; 1.0614x over previous
"""Multi-head self-attention (B=8, N=1024, C=768, H=12) on 8 trn2 NeuronCores.

Sharding: data-parallel over batch — core b computes batch element b end to
end; weights are replicated. No collectives.

Per-core dataflow (all matmuls on TensorE, out = lhsT.T @ rhs, contraction on
the partition dim):

  1. Weights are host-prearranged into dense per-need layouts so every DMA
     descriptor is a contiguous 1.5-2KB line (strided 256B-segment transfers
     crawl at ~60GB/s): wqk [P, t-major (t,k,128)] for the 12 Q/K c'-tiles,
     wv [P, k-major (k,768)], wp [P, k-major (k,768)].  Transfers are issued
     in need order across the three DMA queues (scalar/sync/gpsimd).
  2. qkv^T for Q,K in [c', n] layout as fine-grained units: one 6-matmul
     accumulation run per (c'-tile, 512-col half) into a [128,512] PSUM
     bank, bias fused into the PSUM->SBUF copy on DVE.
  3. V in token-major per-head blocks [ones(64) | V_h] (128 cols per head):
     the 64 ones columns make the A@V matmul produce the softmax row-sums
     replicated across 64 partitions, so normalization needs no partition
     broadcast.  V bias is skipped on-device: since softmax rows sum to 1,
     it folds into an adjusted proj bias bp' = b_proj + W_proj @ b_qkv[V]
     (host-computed).
  4. Per head pair p, slot s -> (mt, g): S^T[m, n] = (K_h^T) @ Q_h^T for
     both heads concurrently in the two PE row-quadrants (K = d = 64). exp
     via ScalarE reading PSUM (3-deep [128,1024] rotation so the PE can run
     ~2 slots ahead of ACT), writing SBUF bf16 (scale folded into the K
     projection host-side; max-subtraction skipped — scores are O(1) and
     softmax is shift-invariant).  The exp table is preloaded via a dummy
     1-col exp during the DMA dead zone.
  5. AV: one 8-matmul consecutive same-bank accumulation run per (head, g)
     into a [128,512] bank from a 2-deep PSUM pool, normalized eagerly
     (custom-DVE fast reciprocal on the replicated sums at PSUM base
     partition 0, then tensor_mul into ouT[c, n]) so the bank frees fast.
  6. proj in two stages: projA = j=0..3 k-tiles -> ysb[nt] (+bias), run
     inside pairs 4-5; tail = j=4,5 + identity-matmul merge of ysb + y DMA
     (bf16, spread over 3 DMA queues).
  7. Pair 5 runs its slots g-major (all g=0 then all g=1) so its AV(g=0),
     normalize, and the nt=0..3 tail units overlap the g=1 exps; only the
     g=1 AV/normalize/tails remain after the last exp.

Scheduling: after each score+exp emission, filler units (each one PSUM-bank
accumulation run + one DVE drain) are drained from a per-pair list by
cumulative time-budget pacing with per-unit earliest-slot constraints
matching DMA arrival and dependency readiness.

All matmul inputs are bf16 (fp32 accumulate); y is written bf16 and upcast
on host.
"""

import numpy as np
import ml_dtypes

B, N, C = 8, 1024, 768
H, D = 12, 64
HB = 2 * D  # per-head V block width: [ones(64) | V_h(64)]
N_CORES = 8
P = 128
KT = C // P  # 6 contraction tiles
NT = N // P  # 8 token tiles
NQT = 2 * C // P  # 12 q/k c'-tiles; pair p uses tiles p and 6+p

_CACHE: dict = {}

MM512 = 215  # ns, warm 512-col bf16 matmul issue-to-issue
MM256 = 110

# x k-tile DMA arrival order (see queue assignment below)
KS_ORDER = [4, 0, 5, 2, 3, 1]
# wv k-chunk arrival order
VORDER = [5, 3, 4, 0, 1, 2]


def _build(cfg: dict):
    import concourse.bass as bass
    import concourse.bacc as bacc
    import concourse.mybir as mybir
    import concourse.tile as tile

    dt = mybir.dt
    f32 = dt.float32
    bf16 = dt.bfloat16

    nc = bacc.Bacc("TRN2", target_bir_lowering=False, debug=False,
                   num_devices=N_CORES)

    xT_d = nc.dram_tensor("xT", [C, N], bf16, kind="ExternalInput")
    wqk_d = nc.dram_tensor("wqk", [P, NQT * KT * P], bf16,
                           kind="ExternalInput")
    wv_d = nc.dram_tensor("wv", [P, KT * C], bf16, kind="ExternalInput")
    wp_d = nc.dram_tensor("wp", [P, KT * C], bf16, kind="ExternalInput")
    bqk_d = nc.dram_tensor("bqk", [P, NQT], f32, kind="ExternalInput")
    bp_d = nc.dram_tensor("bp", [1, C], f32, kind="ExternalInput")
    ident_d = nc.dram_tensor("ident", [P, P], bf16, kind="ExternalInput")
    y_d = nc.dram_tensor("y", [N, C], bf16, kind="ExternalOutput")

    with tile.TileContext(nc, pool_alloc_mode="queue") as tc:
        with (
            tc.tile_pool(name="const", bufs=1) as cpool,
            tc.tile_pool(name="work", bufs=2) as workpool,
            tc.tile_pool(name="ps_s", bufs=3, space="PSUM") as ps_s,
            tc.tile_pool(name="ps_f", bufs=2, space="PSUM") as ps_f,
        ):
            # ---- persistent SBUF inputs ----
            wqk1 = cpool.tile([P, NQT * KT * P], bf16, name="wqk1",
                              tag="wqk1")
            wv1 = cpool.tile([P, KT * C], bf16, name="wv1", tag="wv1")
            wp1 = cpool.tile([P, KT * C], bf16, name="wp1", tag="wp1")
            bqk = cpool.tile([P, NQT], f32, name="bqk", tag="bqk")
            bp = cpool.tile([1, C], f32, name="bp", tag="bp")
            xT1 = cpool.tile([P, KT * N], bf16, name="xT1", tag="xT1")
            ident = cpool.tile([P, P], bf16, name="ident", tag="ident")
            zt = cpool.tile([P, 512], bf16, name="zt", tag="zt")
            nc.vector.memset(zt[:], 0.0)

            def xdma(eng, k):
                eng.dma_start(xT1[:, k * N:(k + 1) * N],
                              xT_d.ap()[k * P:(k + 1) * P, :])

            def tdma(eng, t):
                w = KT * P
                eng.dma_start(wqk1[:, t * w:(t + 1) * w],
                              wqk_d.ap()[:, t * w:(t + 1) * w])

            def vdma(eng, k):
                eng.dma_start(wv1[:, k * C:(k + 1) * C],
                              wv_d.ap()[:, k * C:(k + 1) * C])

            # scalar queue (fast): pair-0 qk tiles, x2/x3/x1, V k0-2, then
            # the exp-table preload (dummy 1-col exp) in the dead zone
            tdma(nc.scalar, 0)
            tdma(nc.scalar, 6)
            xdma(nc.scalar, 2)
            xdma(nc.scalar, 3)
            xdma(nc.scalar, 1)
            vdma(nc.scalar, 0)
            vdma(nc.scalar, 1)
            vdma(nc.scalar, 2)
            dume = cpool.tile([P, 1], bf16, name="dume", tag="dume")
            nc.scalar.activation(dume[:], zt[:, 0:1],
                                 bass.mybir.ActivationFunctionType.Exp)
            # sync queue (slow): x0 + late-deadline qk tiles
            xdma(nc.sync, 0)
            tdma(nc.sync, 1)
            tdma(nc.sync, 7)
            tdma(nc.sync, 5)
            tdma(nc.sync, 11)
            tdma(nc.sync, 3)
            tdma(nc.sync, 9)
            # gpsimd queue: bias, x4/x5, V k5/k3/k4, mid qk tiles, proj wts
            nc.gpsimd.dma_start(bqk[:], bqk_d.ap())
            xdma(nc.gpsimd, 4)
            xdma(nc.gpsimd, 5)
            vdma(nc.gpsimd, 5)
            vdma(nc.gpsimd, 3)
            vdma(nc.gpsimd, 4)
            nc.gpsimd.dma_start(bp[:], bp_d.ap())
            nc.gpsimd.dma_start(ident[:], ident_d.ap())
            tdma(nc.gpsimd, 2)
            tdma(nc.gpsimd, 8)
            tdma(nc.gpsimd, 4)
            tdma(nc.gpsimd, 10)
            nc.gpsimd.dma_start(wp1[:], wp_d.ap())
            bp_b = cpool.tile([P, C], f32, name="bp_b", tag="bp_b")
            nc.gpsimd.partition_broadcast(bp_b[:], bp[:])
            # PE warm-up: junk matmuls during the DMA dead zone so the HAM
            # clock is ramped when real data lands (~13us in).
            jp = ps_s.tile([P, N], f32, name="jp", tag="s")
            for _ in range(15):
                nc.tensor.matmul(jp[:, 0:512], zt[:, 0:P], zt[:],
                                 start=True, stop=True)

            def xT(k):
                return xT1[:, k * N:(k + 1) * N]

            def wqk(t, k):
                return wqk1[:, (t * KT + k) * P:(t * KT + k + 1) * P]

            def wv(k):
                return wv1[:, k * C:(k + 1) * C]

            def wp(k):
                return wp1[:, k * C:(k + 1) * C]

            # ---- persistent SBUF intermediates ----
            qkT = [cpool.tile([P, N], bf16, name=f"qkT{t}", tag=f"qkT{t}")
                   for t in range(NQT)]
            etbig = [cpool.tile([P, NT * 2 * N], bf16, name=f"etbig{i}",
                                tag=f"etbig{i}") for i in range(2)]
            vbig = cpool.tile([P, NT * H * HB], bf16, name="vbig",
                              tag="vbig")
            v = [vbig[:, nt * H * HB:(nt + 1) * H * HB] for nt in range(NT)]
            ouT = [cpool.tile([P, N], bf16, name=f"ouT{j}", tag=f"ouT{j}")
                   for j in range(KT)]
            ysb = [cpool.tile([P, C], bf16, name=f"ysb{nt}", tag=f"ysb{nt}")
                   for nt in range(NT)]

            # ---- fine-grained filler units ----
            def qk_run(t, g):
                # Q^T or K^T tile t, 512-col half g: 6-matmul accumulation
                # run; x k-tiles in DMA arrival order for the early tiles.
                ks = KS_ORDER if t in (0, 1, KT, KT + 1) else list(range(KT))
                pm = ps_f.tile([P, 512], f32, name="fm", tag="f")
                sl = slice(g * 512, (g + 1) * 512)
                for i, k in enumerate(ks):
                    nc.tensor.matmul(pm[:], wqk(t, k), xT(k)[:, sl],
                                     start=(i == 0), stop=(i == KT - 1))
                nc.vector.tensor_scalar_add(qkT[t][:, sl], pm[:],
                                            bqk[:, t:t + 1])

            def v_half(nt, half):
                # V for token tile nt, heads 0-7 (half 0) or 8-11 (half 1)
                dst = v[nt].rearrange("p (h c) -> p h c", c=HB)
                if half == 0:
                    nc.vector.memset(dst[:, :, 0:D], 1.0)
                off, width, h0, h1 = ((0, 512, 0, 8) if half == 0
                                      else (512, 256, 8, 12))
                pm = ps_f.tile([P, 512], f32, name="fm", tag="f")
                for i, k in enumerate(VORDER):
                    nc.tensor.matmul(
                        pm[:, 0:width],
                        xT(k)[:, nt * P:(nt + 1) * P],
                        wv(k)[:, off:off + width],
                        start=(i == 0), stop=(i == KT - 1),
                    )
                srcv = pm[:, 0:width].rearrange("p (h d) -> p h d", d=D)
                nc.vector.tensor_copy(dst[:, h0:h1, D:HB], srcv[:])

            def av_unit(st, par, g):
                # one consecutive 8-matmul accumulation run + eager
                # normalize: sums replicated on partitions 0:64, O^T on
                # 64:128; recip reads PSUM at base partition 0.
                h = 2 * st.p + par
                eb = etbig[st.p % 2]
                pm = ps_f.tile([P, 512], f32, name="fm", tag="f")
                for i in range(NT):
                    s_idx = st.slot_of(i, g)
                    nc.tensor.matmul(
                        pm[:],
                        v[i][:, h * HB:(h + 1) * HB],
                        eb[:, s_idx * N + par * 512:
                           s_idx * N + (par + 1) * 512],
                        start=(i == 0), stop=(i == NT - 1),
                    )
                rb = workpool.tile([D, 512], f32, name="rb", tag="rb")
                nc.vector.reciprocal_approx_fast(rb[:], pm[0:D, :])
                nc.vector.tensor_mul(
                    ouT[st.p][par * D:(par + 1) * D,
                              g * 512:(g + 1) * 512],
                    pm[D:P, :], rb[:])

            def projA(nt, off, width):
                # ysb[nt] <- sum_{j=0..3} ouT[j]^T @ wp[j] + bias
                pm = ps_f.tile([P, 512], f32, name="fm", tag="f")
                for j in range(4):
                    nc.tensor.matmul(
                        pm[:, 0:width],
                        ouT[j][:, nt * P:(nt + 1) * P],
                        wp(j)[:, off:off + width],
                        start=(j == 0), stop=(j == 3),
                    )
                nc.vector.tensor_add(ysb[nt][:, off:off + width],
                                     pm[:, 0:width],
                                     bp_b[:, off:off + width])

            dma_engines = [nc.sync, nc.scalar, nc.gpsimd]

            def tail_unit(nt):
                # j=4,5 contributions + merge with ysb + y DMA. Alternate
                # the merge between PE+ACT (identity mm + copy) and DVE.
                ev = nt % 2 == 0
                pm = ps_f.tile([P, 512], f32, name="fm", tag="f")
                pm2 = ps_f.tile([P, 512], f32, name="fm", tag="f")
                for ps, off, width in ((pm, 0, 512), (pm2, 512, 256)):
                    for j in (4, 5):
                        nc.tensor.matmul(
                            ps[:, 0:width],
                            ouT[j][:, nt * P:(nt + 1) * P],
                            wp(j)[:, off:off + width],
                            start=(j == 4),
                            stop=(j == 5 and not ev),
                        )
                    if ev:
                        nc.tensor.matmul(
                            ps[:, 0:width], ident[:],
                            ysb[nt][:, off:off + width],
                            start=False, stop=True)
                yb = workpool.tile([P, C], bf16, name="yb", tag="yb",
                                   bufs=4)
                if ev:
                    nc.scalar.copy(yb[:, 0:512], pm[:])
                    nc.scalar.copy(yb[:, 512:C], pm2[:, 0:256])
                else:
                    nc.vector.tensor_add(yb[:, 0:512], pm[:],
                                         ysb[nt][:, 0:512])
                    nc.vector.tensor_add(yb[:, 512:C], pm2[:, 0:256],
                                         ysb[nt][:, 512:C])
                dma_engines[nt % 3].dma_start(
                    y_d.ap()[nt * P:(nt + 1) * P, :], yb[:])

            # ---- attention pieces ----
            class PairState:
                def __init__(self, p):
                    self.p = p
                    self.gmajor = (p == H // 2 - 1)

                def slot_of(self, mt, g):
                    return g * NT + mt if self.gmajor else 2 * mt + g

            def score_exp(st, mt, g):
                qt = qkT[st.p]
                kt = qkT[NQT // 2 + st.p]
                sp = ps_s.tile([P, N], f32, name="sp", tag="s")
                for par in range(2):
                    o = par * D
                    nc.tensor.matmul(
                        sp[:, par * 512:(par + 1) * 512],
                        kt[o:o + D, mt * P:(mt + 1) * P],
                        qt[o:o + D, g * 512:(g + 1) * 512],
                        start=True, stop=True,
                    )
                s_idx = st.slot_of(mt, g)
                et = etbig[st.p % 2][:, s_idx * N:(s_idx + 1) * N]
                nc.scalar.activation(
                    et, sp[:], bass.mybir.ActivationFunctionType.Exp)

            # ---- per-pair filler unit lists: (min_slot, cost_ns, fn) ----
            NPAIR = H // 2
            units: list = [[] for _ in range(NPAIR)]
            UQK = 6 * MM512 + 80
            UV0 = 6 * MM512 + 80
            UV1 = 6 * MM256 + 80
            UAV = 8 * MM512 + 80
            UPA = {512: 4 * MM512 + 80, 256: 4 * MM256 + 80}
            UTLE = 3 * MM512 + 3 * MM256 + 80
            UTLO = 2 * MM512 + 2 * MM256 + 80

            def add(p, ms, cost, fn):
                units[p].append((ms, cost, fn))

            # pair 0: pair-1 qk tiles early (t1/t7 land ~14-18us), V half0
            # as the wv chunks land, vh1 x2 late.
            add(0, 0, UQK, lambda: qk_run(1, 0))
            add(0, 1, UQK, lambda: qk_run(7, 0))
            add(0, 2, UQK, lambda: qk_run(1, 1))
            add(0, 3, UQK, lambda: qk_run(7, 1))
            for nt in range(NT):
                add(0, nt + 2, UV0, lambda nt=nt: v_half(nt, 0))
            # pairs 1-4: AV(prev) x4 from slot 2, hosted qk tiles, vh1,
            # projA in pair 4.
            for p in range(1, 5):
                for i, (par, g) in enumerate(
                        ((0, 0), (1, 0), (0, 1), (1, 1))):
                    add(p, 2 + i, UAV,
                        lambda par=par, g=g: ("av_prev", par, g))
            for i, (t, g) in enumerate(((2, 0), (8, 0), (2, 1), (8, 1))):
                add(1, 6 + i, UQK, lambda t=t, g=g: qk_run(t, g))
            for nt in (0, 1, 2):
                add(1, 0, UV1, lambda nt=nt: v_half(nt, 1))
            add(1, 10, UQK, lambda: qk_run(5, 0))
            for i, (t, g) in enumerate(((3, 0), (9, 0), (3, 1), (9, 1))):
                add(2, 6 + i, UQK, lambda t=t, g=g: qk_run(t, g))
            for nt in (3, 4, 5):
                add(2, 0, UV1, lambda nt=nt: v_half(nt, 1))
            add(2, 10, UQK, lambda: qk_run(5, 1))
            for i, (t, g) in enumerate(((4, 0), (10, 0), (4, 1), (10, 1))):
                add(3, 6 + i, UQK, lambda t=t, g=g: qk_run(t, g))
            for nt in (6, 7):
                add(3, 0, UV1, lambda nt=nt: v_half(nt, 1))
            add(3, 10, UQK, lambda: qk_run(11, 0))
            add(3, 11, UQK, lambda: qk_run(11, 1))
            for nt in range(6):
                add(4, 7, UPA[512], lambda nt=nt: projA(nt, 0, 512))
                add(4, 7, UPA[256], lambda nt=nt: projA(nt, 512, 256))
            # pair 5 (g-major slots): AV(p4) x4, projA nt 6-7, AV(p5,g0)
            # after slot 9, then the nt 0-3 tails.
            for i, (par, g) in enumerate(((0, 0), (1, 0), (0, 1), (1, 1))):
                add(5, 2 + i, UAV, lambda par=par, g=g: ("av_prev", par, g))
            for nt in (6, 7):
                add(5, 2, UPA[512], lambda nt=nt: projA(nt, 0, 512))
                add(5, 2, UPA[256], lambda nt=nt: projA(nt, 512, 256))
            add(5, 10, UAV, lambda: ("av_cur", 0, 0))
            add(5, 11, UAV, lambda: ("av_cur", 1, 0))
            for nt in range(4):
                add(5, 13, UTLE if nt % 2 == 0 else UTLO,
                    lambda nt=nt: tail_unit(nt))

            # pair-0 q/k tiles up front — g0 halves first so the first
            # score slot unblocks as early as possible
            qk_run(0, 0)
            qk_run(6, 0)
            qk_run(0, 1)
            qk_run(6, 1)

            # ---- main loop ----
            prev = None
            for p in range(NPAIR):
                cur = PairState(p)
                ulist = units[p]
                total = sum(c for _, c, _ in ulist) + 16 * MM512
                spent = 0

                def emit(fn):
                    r = fn()
                    if isinstance(r, tuple):
                        av_unit(prev if r[0] == "av_prev" else cur,
                                r[1], r[2])
                for s in range(16):
                    if cur.gmajor:
                        g, mt = divmod(s, NT)
                    else:
                        mt, g = divmod(s, 2)
                    score_exp(cur, mt, g)
                    spent += MM512
                    budget = total * (s + 1) // 16
                    while ulist:
                        idx = next((i for i, u in enumerate(ulist)
                                    if u[0] <= s), None)
                        if idx is None or spent > budget:
                            break
                        _, c, fn = ulist.pop(idx)
                        emit(fn)
                        spent += c
                for _, c, fn in ulist:
                    emit(fn)
                prev = cur

            # ---- tail: pair-5 g=1 AV + normalize + nt 4-7 tails ----
            for par in range(2):
                av_unit(prev, par, 1)
            for nt in range(4, NT):
                tail_unit(nt)

    nc.compile()
    return nc


DEFAULT_CFG = dict()


def _host_prep(x, W_qkv, b_qkv, W_proj, b_proj, cfg):
    """Shard + lay out host-side numpy inputs per core."""
    scale = np.float32(1.0 / np.sqrt(D))
    wqkvT = np.ascontiguousarray(W_qkv.T).astype(np.float32)
    # fold the 1/sqrt(D) score scale into the K projection (cols C:2C)
    wqkvT[:, C:2 * C] *= scale
    wqkvT = wqkvT.astype(ml_dtypes.bfloat16)
    # dense per-need layouts (one contiguous 1.5KB line per partition row
    # per transfer chunk):
    # wqk[p, (t,k,128)]: c'-tile-major Q/K weights
    wqk = np.empty((P, NQT * KT * P), dtype=ml_dtypes.bfloat16)
    for t in range(NQT):
        for k in range(KT):
            blk = wqkvT[k * P:(k + 1) * P, t * P:(t + 1) * P]
            wqk[:, (t * KT + k) * P:(t * KT + k + 1) * P] = blk
    # wv[p, (k,768)]: V weights k-major
    wv = np.empty((P, KT * C), dtype=ml_dtypes.bfloat16)
    for k in range(KT):
        wv[:, k * C:(k + 1) * C] = wqkvT[k * P:(k + 1) * P, 2 * C:3 * C]
    # wp[p, (k,768)]: proj weights k-major
    wprojT = np.ascontiguousarray(W_proj.T).astype(ml_dtypes.bfloat16)
    wp = np.empty((P, KT * C), dtype=ml_dtypes.bfloat16)
    for k in range(KT):
        wp[:, k * C:(k + 1) * C] = wprojT[k * P:(k + 1) * P, :]
    bqk_f = b_qkv[:2 * C].astype(np.float32).copy()
    bqk_f[C:2 * C] *= scale
    bqk = np.ascontiguousarray(bqk_f.reshape(NQT, P).T).astype(np.float32)
    bp_eff = (b_proj.astype(np.float64)
              + W_proj.astype(np.float64) @ b_qkv[2 * C:].astype(np.float64))
    bp = bp_eff.astype(np.float32).reshape(1, C)
    ident = np.eye(P, dtype=ml_dtypes.bfloat16)
    in_maps = []
    for b in range(N_CORES):
        xT = np.ascontiguousarray(x[b].T).astype(ml_dtypes.bfloat16)
        in_maps.append({"xT": xT, "wqk": wqk, "wv": wv, "wp": wp,
                        "bqk": bqk, "bp": bp, "ident": ident})
    return in_maps


def get_nc(cfg=None):
    cfg = dict(DEFAULT_CFG, **(cfg or {}))
    key = tuple(sorted(cfg.items()))
    if key not in _CACHE:
        _CACHE[key] = _build(cfg)
    return _CACHE[key]


def run(inputs, cfg=None, **run_kwargs):
    from concourse import bass_utils

    cfg = dict(DEFAULT_CFG, **(cfg or {}))
    nc = get_nc(cfg)
    in_maps = _host_prep(inputs["x"], inputs["W_qkv"], inputs["b_qkv"],
                         inputs["W_proj"], inputs["b_proj"], cfg)
    res = bass_utils.run_bass_kernel_spmd(
        nc, in_maps, core_ids=list(range(N_CORES)), **run_kwargs)
    out = np.stack([res.results[b]["y"].astype(np.float32)
                    for b in range(N_CORES)], axis=0)
    return out, res


def kernel(**inputs) -> np.ndarray:
    inputs = {k: np.asarray(v) for k, v in inputs.items()}
    out, _ = run(inputs)
    return out


# revision 14
# speedup vs baseline: 1.1145x; 1.0097x over previous
"""Multi-head self-attention (B=8, N=1024, C=768, H=12) on 8 trn2 NeuronCores.

Sharding: data-parallel over batch — core b computes batch element b end to
end; weights are replicated. No collectives.

Per-core dataflow (all matmuls on TensorE, out = lhsT.T @ rhs, contraction on
the partition dim):

  1. Weights are host-prearranged into dense per-need layouts so every DMA
     descriptor is a contiguous 1.5-2KB line (strided 256B-segment transfers
     crawl at ~60GB/s): wqk [P, t-major (t,k,128)] for the 12 Q/K c'-tiles,
     wv [P, k-major (k,768)], wp [P, k-major (k,768)].  Transfers are issued
     in need order across the three DMA queues (scalar/sync/gpsimd).
  2. qkv^T for Q,K in [c', n] layout as fine-grained units: one 6-matmul
     accumulation run per (c'-tile, 512-col half) into a [128,512] PSUM
     bank, bias fused into the PSUM->SBUF copy on DVE.
  3. V in token-major per-head blocks [ones(64) | V_h] (128 cols per head):
     the 64 ones columns make the A@V matmul produce the softmax row-sums
     replicated across 64 partitions, so normalization needs no partition
     broadcast.  V bias is skipped on-device: since softmax rows sum to 1,
     it folds into an adjusted proj bias bp' = b_proj + W_proj @ b_qkv[V]
     (host-computed).
  4. Per head pair p, slot s -> (mt, g): S^T[m, n] = (K_h^T) @ Q_h^T for
     both heads concurrently in the two PE row-quadrants (K = d = 64). exp
     via ScalarE reading PSUM (3-deep [128,1024] rotation so the PE can run
     ~2 slots ahead of ACT), writing SBUF bf16 (scale folded into the K
     projection host-side; max-subtraction skipped — scores are O(1) and
     softmax is shift-invariant).  The exp table is preloaded via a dummy
     1-col exp during the DMA dead zone.
  5. AV: one 8-matmul consecutive same-bank accumulation run per (head, g)
     into a [128,512] bank from a 2-deep PSUM pool, normalized eagerly
     (custom-DVE fast reciprocal on the replicated sums at PSUM base
     partition 0, then tensor_mul into ouT[c, n]) so the bank frees fast.
  6. proj in two stages: projA = j=0..3 k-tiles -> ysb[nt] (+bias), run
     inside pairs 4-5; tail = j=4,5 + identity-matmul merge of ysb + y DMA
     (bf16, spread over 3 DMA queues).
  7. Pair 5 runs its slots g-major (all g=0 then all g=1) so its AV(g=0),
     normalize, and the nt=0..3 tail units overlap the g=1 exps; only the
     g=1 AV/normalize/tails remain after the last exp.

Scheduling: after each score+exp emission, filler units (each one PSUM-bank
accumulation run + one DVE drain) are drained from a per-pair list by
cumulative time-budget pacing with per-unit earliest-slot constraints
matching DMA arrival and dependency readiness.

All matmul inputs are bf16 (fp32 accumulate); y is written bf16 and upcast
on host.
"""

import numpy as np
import ml_dtypes

B, N, C = 8, 1024, 768
H, D = 12, 64
HB = 2 * D  # per-head V block width: [ones(64) | V_h(64)]
N_CORES = 8
P = 128
KT = C // P  # 6 contraction tiles
NT = N // P  # 8 token tiles
NQT = 2 * C // P  # 12 q/k c'-tiles; pair p uses tiles p and 6+p

_CACHE: dict = {}

MM512 = 215  # ns, warm 512-col bf16 matmul issue-to-issue
MM256 = 110

# x k-tile DMA arrival order (see queue assignment below)
KS_ORDER = [4, 0, 2, 5, 1, 3]
# wv k-chunk arrival order
VORDER = [5, 3, 0, 4, 1, 2]


def _build(cfg: dict):
    import concourse.bass as bass
    import concourse.bacc as bacc
    import concourse.mybir as mybir
    import concourse.tile as tile

    dt = mybir.dt
    f32 = dt.float32
    bf16 = dt.bfloat16

    nc = bacc.Bacc("TRN2", target_bir_lowering=False, debug=False,
                   num_devices=N_CORES)

    xT_d = nc.dram_tensor("xT", [C, N], bf16, kind="ExternalInput")
    wqk_d = nc.dram_tensor("wqk", [P, NQT * KT * P], bf16,
                           kind="ExternalInput")
    wv_d = nc.dram_tensor("wv", [P, KT * C], bf16, kind="ExternalInput")
    wp_d = nc.dram_tensor("wp", [P, KT * C], bf16, kind="ExternalInput")
    bqk_d = nc.dram_tensor("bqk", [P, NQT], f32, kind="ExternalInput")
    bp_d = nc.dram_tensor("bp", [1, C], f32, kind="ExternalInput")
    ident_d = nc.dram_tensor("ident", [P, P], bf16, kind="ExternalInput")
    y_d = nc.dram_tensor("y", [N, C], bf16, kind="ExternalOutput")

    with tile.TileContext(nc, pool_alloc_mode="queue") as tc:
        with (
            tc.tile_pool(name="const", bufs=1) as cpool,
            tc.tile_pool(name="work", bufs=2) as workpool,
            tc.tile_pool(name="ps_s", bufs=3, space="PSUM") as ps_s,
            tc.tile_pool(name="ps_f", bufs=2, space="PSUM") as ps_f,
        ):
            # ---- persistent SBUF inputs ----
            wqk1 = cpool.tile([P, NQT * KT * P], bf16, name="wqk1",
                              tag="wqk1")
            wv1 = cpool.tile([P, KT * C], bf16, name="wv1", tag="wv1")
            wp1 = cpool.tile([P, KT * C], bf16, name="wp1", tag="wp1")
            bqk = cpool.tile([P, NQT], f32, name="bqk", tag="bqk")
            bp = cpool.tile([1, C], f32, name="bp", tag="bp")
            xT1 = cpool.tile([P, KT * N], bf16, name="xT1", tag="xT1")
            ident = cpool.tile([P, P], bf16, name="ident", tag="ident")
            zt = cpool.tile([P, 512], bf16, name="zt", tag="zt")
            nc.vector.memset(zt[:], 0.0)

            def xdma(eng, k):
                eng.dma_start(xT1[:, k * N:(k + 1) * N],
                              xT_d.ap()[k * P:(k + 1) * P, :])

            def tdma(eng, t):
                w = KT * P
                eng.dma_start(wqk1[:, t * w:(t + 1) * w],
                              wqk_d.ap()[:, t * w:(t + 1) * w])

            def vdma(eng, k):
                eng.dma_start(wv1[:, k * C:(k + 1) * C],
                              wv_d.ap()[:, k * C:(k + 1) * C])

            # scalar queue (fast): t0/t6 + two x tiles, then the exp-table
            # preload (dummy 1-col exp) in the dead zone, then V k0-2
            tdma(nc.scalar, 0)
            xdma(nc.scalar, 2)
            tdma(nc.scalar, 6)
            xdma(nc.scalar, 3)
            dume = cpool.tile([P, 1], bf16, name="dume", tag="dume")
            nc.scalar.activation(dume[:], zt[:, 0:1],
                                 bass.mybir.ActivationFunctionType.Exp)
            vdma(nc.scalar, 0)
            vdma(nc.scalar, 1)
            vdma(nc.scalar, 2)
            # sync queue (slow): x0/x1 + late-deadline qk tiles
            xdma(nc.sync, 0)
            xdma(nc.sync, 1)
            tdma(nc.sync, 1)
            tdma(nc.sync, 7)
            tdma(nc.sync, 5)
            tdma(nc.sync, 11)
            tdma(nc.sync, 3)
            tdma(nc.sync, 9)
            # gpsimd queue: bias, x4/x5, V k5/k3/k4, mid qk tiles, proj wts
            nc.gpsimd.dma_start(bqk[:], bqk_d.ap())
            xdma(nc.gpsimd, 4)
            xdma(nc.gpsimd, 5)
            vdma(nc.gpsimd, 5)
            vdma(nc.gpsimd, 3)
            vdma(nc.gpsimd, 4)
            nc.gpsimd.dma_start(bp[:], bp_d.ap())
            nc.gpsimd.dma_start(ident[:], ident_d.ap())
            tdma(nc.gpsimd, 2)
            tdma(nc.gpsimd, 8)
            tdma(nc.gpsimd, 4)
            tdma(nc.gpsimd, 10)
            nc.gpsimd.dma_start(wp1[:], wp_d.ap())
            bp_b = cpool.tile([P, C], f32, name="bp_b", tag="bp_b")
            nc.gpsimd.partition_broadcast(bp_b[:], bp[:])
            # PE warm-up: junk matmuls during the DMA dead zone so the HAM
            # clock is ramped when real data lands (~13us in).
            jp = ps_s.tile([P, N], f32, name="jp", tag="s")
            for _ in range(15):
                nc.tensor.matmul(jp[:, 0:512], zt[:, 0:P], zt[:],
                                 start=True, stop=True)

            def xT(k):
                return xT1[:, k * N:(k + 1) * N]

            def wqk(t, k):
                return wqk1[:, (t * KT + k) * P:(t * KT + k + 1) * P]

            def wv(k):
                return wv1[:, k * C:(k + 1) * C]

            def wp(k):
                return wp1[:, k * C:(k + 1) * C]

            # ---- persistent SBUF intermediates ----
            qkT = [cpool.tile([P, N], bf16, name=f"qkT{t}", tag=f"qkT{t}")
                   for t in range(NQT)]
            etbig = [cpool.tile([P, NT * 2 * N], bf16, name=f"etbig{i}",
                                tag=f"etbig{i}") for i in range(2)]
            vbig = cpool.tile([P, NT * H * HB], bf16, name="vbig",
                              tag="vbig")
            v = [vbig[:, nt * H * HB:(nt + 1) * H * HB] for nt in range(NT)]
            ouT = [cpool.tile([P, N], bf16, name=f"ouT{j}", tag=f"ouT{j}")
                   for j in range(KT)]
            ysb = [cpool.tile([P, C], bf16, name=f"ysb{nt}", tag=f"ysb{nt}")
                   for nt in range(NT)]

            # ---- fine-grained filler units ----
            def qk_run(t, g, warm=False):
                # Q^T or K^T tile t, 512-col half g: 6-matmul accumulation
                # run; x k-tiles in DMA arrival order for the early tiles.
                # warm=True interleaves junk matmuls after each x-gated
                # member so the PE HAM clock stays ramped across the x-tile
                # DMA arrival gaps.
                ks = KS_ORDER if t in (0, 1, KT, KT + 1) else list(range(KT))
                pm = ps_f.tile([P, 512], f32, name="fm", tag="f")
                sl = slice(g * 512, (g + 1) * 512)
                for i, k in enumerate(ks):
                    nc.tensor.matmul(pm[:], wqk(t, k), xT(k)[:, sl],
                                     start=(i == 0), stop=(i == KT - 1))
                    if warm and i < KT - 1:
                        for _ in range(3):
                            nc.tensor.matmul(jp[:, 0:512], zt[:, 0:P],
                                             zt[:], start=True, stop=True)
                nc.vector.tensor_scalar_add(qkT[t][:, sl], pm[:],
                                            bqk[:, t:t + 1])

            def v_half(nt, half):
                # V for token tile nt, heads 0-7 (half 0) or 8-11 (half 1)
                dst = v[nt].rearrange("p (h c) -> p h c", c=HB)
                if half == 0:
                    nc.vector.memset(dst[:, :, 0:D], 1.0)
                off, width, h0, h1 = ((0, 512, 0, 8) if half == 0
                                      else (512, 256, 8, 12))
                pm = ps_f.tile([P, 512], f32, name="fm", tag="f")
                for i, k in enumerate(VORDER):
                    nc.tensor.matmul(
                        pm[:, 0:width],
                        xT(k)[:, nt * P:(nt + 1) * P],
                        wv(k)[:, off:off + width],
                        start=(i == 0), stop=(i == KT - 1),
                    )
                srcv = pm[:, 0:width].rearrange("p (h d) -> p h d", d=D)
                nc.vector.tensor_copy(dst[:, h0:h1, D:HB], srcv[:])

            def av_unit(st, par, g):
                # one consecutive 8-matmul accumulation run + eager
                # normalize: sums replicated on partitions 0:64, O^T on
                # 64:128; recip reads PSUM at base partition 0.
                h = 2 * st.p + par
                eb = etbig[st.p % 2]
                pm = ps_f.tile([P, 512], f32, name="fm", tag="f")
                for i in range(NT):
                    s_idx = st.slot_of(i, g)
                    nc.tensor.matmul(
                        pm[:],
                        v[i][:, h * HB:(h + 1) * HB],
                        eb[:, s_idx * N + par * 512:
                           s_idx * N + (par + 1) * 512],
                        start=(i == 0), stop=(i == NT - 1),
                    )
                rb = workpool.tile([D, 512], f32, name="rb", tag="rb")
                nc.vector.reciprocal_approx_fast(rb[:], pm[0:D, :])
                nc.vector.tensor_mul(
                    ouT[st.p][par * D:(par + 1) * D,
                              g * 512:(g + 1) * 512],
                    pm[D:P, :], rb[:])

            def projA(nt, off, width):
                # ysb[nt] <- sum_{j=0..3} ouT[j]^T @ wp[j] + bias
                pm = ps_f.tile([P, 512], f32, name="fm", tag="f")
                for j in range(4):
                    nc.tensor.matmul(
                        pm[:, 0:width],
                        ouT[j][:, nt * P:(nt + 1) * P],
                        wp(j)[:, off:off + width],
                        start=(j == 0), stop=(j == 3),
                    )
                nc.vector.tensor_add(ysb[nt][:, off:off + width],
                                     pm[:, 0:width],
                                     bp_b[:, off:off + width])

            dma_engines = [nc.sync, nc.scalar, nc.gpsimd]

            def tail_unit(nt):
                # j=4,5 contributions + DVE merge with ysb + y DMA (ACT is
                # exp-saturated at the tail, so drains go to DVE only).
                pm = ps_f.tile([P, 512], f32, name="fm", tag="f")
                pm2 = ps_f.tile([P, 512], f32, name="fm", tag="f")
                for ps, off, width in ((pm, 0, 512), (pm2, 512, 256)):
                    for j in (4, 5):
                        nc.tensor.matmul(
                            ps[:, 0:width],
                            ouT[j][:, nt * P:(nt + 1) * P],
                            wp(j)[:, off:off + width],
                            start=(j == 4), stop=(j == 5),
                        )
                yb = workpool.tile([P, C], bf16, name="yb", tag="yb",
                                   bufs=4)
                nc.vector.tensor_add(yb[:, 0:512], pm[:],
                                     ysb[nt][:, 0:512])
                nc.vector.tensor_add(yb[:, 512:C], pm2[:, 0:256],
                                     ysb[nt][:, 512:C])
                dma_engines[nt % 3].dma_start(
                    y_d.ap()[nt * P:(nt + 1) * P, :], yb[:])

            # ---- attention pieces ----
            class PairState:
                def __init__(self, p):
                    self.p = p
                    self.gmajor = (p == H // 2 - 1)

                def slot_of(self, mt, g):
                    return g * NT + mt if self.gmajor else 2 * mt + g

            def score_exp(st, mt, g):
                qt = qkT[st.p]
                kt = qkT[NQT // 2 + st.p]
                sp = ps_s.tile([P, N], f32, name="sp", tag="s")
                for par in range(2):
                    o = par * D
                    nc.tensor.matmul(
                        sp[:, par * 512:(par + 1) * 512],
                        kt[o:o + D, mt * P:(mt + 1) * P],
                        qt[o:o + D, g * 512:(g + 1) * 512],
                        start=True, stop=True,
                    )
                s_idx = st.slot_of(mt, g)
                et = etbig[st.p % 2][:, s_idx * N:(s_idx + 1) * N]
                nc.scalar.activation(
                    et, sp[:], bass.mybir.ActivationFunctionType.Exp)

            # ---- per-pair filler unit lists: (min_slot, cost_ns, fn) ----
            NPAIR = H // 2
            units: list = [[] for _ in range(NPAIR)]
            UQK = 6 * MM512 + 80
            UV0 = 6 * MM512 + 80
            UV1 = 6 * MM256 + 80
            UAV = 8 * MM512 + 80
            UPA = {512: 4 * MM512 + 80, 256: 4 * MM256 + 80}
            UTL = 2 * MM512 + 2 * MM256 + 80

            def add(p, ms, cost, fn):
                units[p].append((ms, cost, fn))

            # pair 0: pair-1 qk tiles early (t1/t7 land ~14-18us), V half0
            # as the wv chunks land, vh1 x2 late.
            add(0, 1, UQK, lambda: qk_run(1, 0))
            add(0, 3, UQK, lambda: qk_run(7, 0))
            add(0, 4, UQK, lambda: qk_run(1, 1))
            add(0, 5, UQK, lambda: qk_run(7, 1))
            for nt in range(NT):
                add(0, nt + 3, UV0, lambda nt=nt: v_half(nt, 0))
            # pairs 1-4: AV(prev) x4 from slot 2, hosted qk tiles, vh1,
            # projA in pair 4.
            for p in range(1, 5):
                for i, (par, g) in enumerate(
                        ((0, 0), (1, 0), (0, 1), (1, 1))):
                    add(p, 2 + i, UAV,
                        lambda par=par, g=g: ("av_prev", par, g))
            for i, (t, g) in enumerate(((2, 0), (8, 0), (2, 1), (8, 1))):
                add(1, 6 + i, UQK, lambda t=t, g=g: qk_run(t, g))
            for nt in (0, 1, 2):
                add(1, 0, UV1, lambda nt=nt: v_half(nt, 1))
            add(1, 10, UQK, lambda: qk_run(5, 0))
            for i, (t, g) in enumerate(((3, 0), (9, 0), (3, 1), (9, 1))):
                add(2, 6 + i, UQK, lambda t=t, g=g: qk_run(t, g))
            for nt in (3, 4, 5):
                add(2, 0, UV1, lambda nt=nt: v_half(nt, 1))
            add(2, 10, UQK, lambda: qk_run(5, 1))
            for i, (t, g) in enumerate(((4, 0), (10, 0), (4, 1), (10, 1))):
                add(3, 6 + i, UQK, lambda t=t, g=g: qk_run(t, g))
            for nt in (6, 7):
                add(3, 0, UV1, lambda nt=nt: v_half(nt, 1))
            add(3, 10, UQK, lambda: qk_run(11, 0))
            add(3, 11, UQK, lambda: qk_run(11, 1))
            for nt in range(6):
                add(4, 7, UPA[512], lambda nt=nt: projA(nt, 0, 512))
                add(4, 7, UPA[256], lambda nt=nt: projA(nt, 512, 256))
            # pair 5 (g-major slots): AV(p4) x4, projA nt 6-7, AV(p5,g0)
            # after slot 9, then the nt 0-3 tails.
            for i, (par, g) in enumerate(((0, 0), (1, 0), (0, 1), (1, 1))):
                add(5, 2 + i, UAV, lambda par=par, g=g: ("av_prev", par, g))
            for nt in (6, 7):
                add(5, 2, UPA[512], lambda nt=nt: projA(nt, 0, 512))
                add(5, 2, UPA[256], lambda nt=nt: projA(nt, 512, 256))
            add(5, 10, UAV, lambda: ("av_cur", 0, 0))
            add(5, 11, UAV, lambda: ("av_cur", 1, 0))
            for nt in range(4):
                add(5, 13, UTL, lambda nt=nt: tail_unit(nt))

            # pair-0 q/k tiles up front — g0 halves first so the first
            # score slot unblocks as early as possible; the first two are
            # warm (junk-interleaved) to bridge the x-arrival gaps.
            qk_run(0, 0, warm=True)
            qk_run(6, 0, warm=True)
            qk_run(0, 1)
            qk_run(6, 1)

            # ---- main loop ----
            prev = None
            for p in range(NPAIR):
                cur = PairState(p)
                ulist = units[p]
                total = sum(c for _, c, _ in ulist) + 16 * MM512
                spent = 0

                def emit(fn):
                    r = fn()
                    if isinstance(r, tuple):
                        av_unit(prev if r[0] == "av_prev" else cur,
                                r[1], r[2])
                for s in range(16):
                    if cur.gmajor:
                        g, mt = divmod(s, NT)
                    else:
                        mt, g = divmod(s, 2)
                    score_exp(cur, mt, g)
                    spent += MM512
                    budget = total * (s + 1) // 16
                    while ulist:
                        idx = next((i for i, u in enumerate(ulist)
                                    if u[0] <= s), None)
                        if idx is None or spent > budget:
                            break
                        _, c, fn = ulist.pop(idx)
                        emit(fn)
                        spent += c
                for _, c, fn in ulist:
                    emit(fn)
                prev = cur

            # ---- tail: pair-5 g=1 AV + normalize + nt 4-7 tails ----
            for par in range(2):
                av_unit(prev, par, 1)
            for nt in range(4, NT):
                tail_unit(nt)

    nc.compile()
    return nc


DEFAULT_CFG = dict()


def _host_prep(x, W_qkv, b_qkv, W_proj, b_proj, cfg):
    """Shard + lay out host-side numpy inputs per core."""
    scale = np.float32(1.0 / np.sqrt(D))
    wqkvT = np.ascontiguousarray(W_qkv.T).astype(np.float32)
    # fold the 1/sqrt(D) score scale into the K projection (cols C:2C)
    wqkvT[:, C:2 * C] *= scale
    wqkvT = wqkvT.astype(ml_dtypes.bfloat16)
    # dense per-need layouts (one contiguous 1.5KB line per partition row
    # per transfer chunk):
    # wqk[p, (t,k,128)]: c'-tile-major Q/K weights
    wqk = np.empty((P, NQT * KT * P), dtype=ml_dtypes.bfloat16)
    for t in range(NQT):
        for k in range(KT):
            blk = wqkvT[k * P:(k + 1) * P, t * P:(t + 1) * P]
            wqk[:, (t * KT + k) * P:(t * KT + k + 1) * P] = blk
    # wv[p, (k,768)]: V weights k-major
    wv = np.empty((P, KT * C), dtype=ml_dtypes.bfloat16)
    for k in range(KT):
        wv[:, k * C:(k + 1) * C] = wqkvT[k * P:(k + 1) * P, 2 * C:3 * C]
    # wp[p, (k,768)]: proj weights k-major
    wprojT = np.ascontiguousarray(W_proj.T).astype(ml_dtypes.bfloat16)
    wp = np.empty((P, KT * C), dtype=ml_dtypes.bfloat16)
    for k in range(KT):
        wp[:, k * C:(k + 1) * C] = wprojT[k * P:(k + 1) * P, :]
    bqk_f = b_qkv[:2 * C].astype(np.float32).copy()
    bqk_f[C:2 * C] *= scale
    bqk = np.ascontiguousarray(bqk_f.reshape(NQT, P).T).astype(np.float32)
    bp_eff = (b_proj.astype(np.float64)
              + W_proj.astype(np.float64) @ b_qkv[2 * C:].astype(np.float64))
    bp = bp_eff.astype(np.float32).reshape(1, C)
    ident = np.eye(P, dtype=ml_dtypes.bfloat16)
    in_maps = []
    for b in range(N_CORES):
        xT = np.ascontiguousarray(x[b].T).astype(ml_dtypes.bfloat16)
        in_maps.append({"xT": xT, "wqk": wqk, "wv": wv, "wp": wp,
                        "bqk": bqk, "bp": bp, "ident": ident})
    return in_maps


def get_nc(cfg=None):
    cfg = dict(DEFAULT_CFG, **(cfg or {}))
    key = tuple(sorted(cfg.items()))
    if key not in _CACHE:
        _CACHE[key] = _build(cfg)
    return _CACHE[key]


def run(inputs, cfg=None, **run_kwargs):
    from concourse import bass_utils

    cfg = dict(DEFAULT_CFG, **(cfg or {}))
    nc = get_nc(cfg)
    in_maps = _host_prep(inputs["x"], inputs["W_qkv"], inputs["b_qkv"],
                         inputs["W_proj"], inputs["b_proj"], cfg)
    res = bass_utils.run_bass_kernel_spmd(
        nc, in_maps, core_ids=list(range(N_CORES)), **run_kwargs)
    out = np.stack([res.results[b]["y"].astype(np.float32)
                    for b in range(N_CORES)], axis=0)
    return out, res


def kernel(**inputs) -> np.ndarray:
    inputs = {k: np.asarray(v) for k, v in inputs.items()}
    out, _ = run(inputs)
    return out
